# revision 1
# baseline (speedup 1.0000x reference)
"""Trainium2 Bass kernel for nn_CausalFullAttention_13735305413109.

Causal attention with a data-dependent cumprod decay gate and no softmax.
Because there is no softmax, the masked quadratic attention is algebraically
a chunked linear attention:
    out_i = q'_i @ State_{blk(i)} + sum_{j<=i, same blk} (q'_i.k'_j) v_j
    State_t = sum_{j < t*BLK} k'_j (x) v_j
with q' = q*SCALE*a_cum, k' = k/max(a_cum,1e-8), computed per (batch, head).

Sharding: head-parallel across 8 cores (head h -> core h, both batches local),
no cross-device communication; each core emits its partial output projection
out_h @ w_out[h*64:(h+1)*64, :] over all 4096 token rows, and the host sums
the 8 partials (+ b_out) as the unshard step.
"""
import numpy as np
from contextlib import ExitStack

import concourse.bass as bass
import concourse.bacc as bacc
import concourse.mybir as mybir
import concourse.tile as tile
from concourse.bass_utils import run_bass_kernel_spmd

F32 = mybir.dt.float32
AF = mybir.ActivationFunctionType
ALU = mybir.AluOpType

B = 2
N = 2048
DIM = 512
HEADS = 8
DH = 64
NTOK = B * N            # 4096 token rows
BLK = 128               # token block
NBLK = N // BLK         # 16 blocks per batch
PANEL = 512             # projection panel (moving free dim)
NPAN = NTOK // PANEL    # 8 panels
NCHUNK = DIM // 128     # 4 contraction chunks
SCALE = DH ** -0.5
LOG_SQRT_DIM = float(np.log(np.sqrt(DIM)))
EPS_INV = 1e-8


def build_nc(with_qkv_bias: bool):
    nc = bacc.Bacc()
    xT_d = nc.dram_tensor("xT", [DIM, NTOK], F32, kind="ExternalInput")
    wqk_d = nc.dram_tensor("wqk", [128, NCHUNK, 128], F32, kind="ExternalInput")
    wvz_d = nc.dram_tensor("wvz", [128, NCHUNK, 128], F32, kind="ExternalInput")
    wout_d = nc.dram_tensor("wout", [DH, DIM], F32, kind="ExternalInput")
    ba_d = nc.dram_tensor("ba", [128, 1], F32, kind="ExternalInput")
    nba_d = nc.dram_tensor("nba", [128, 1], F32, kind="ExternalInput")
    ident_d = nc.dram_tensor("ident", [128, 128], F32, kind="ExternalInput")
    mask_d = nc.dram_tensor("mask", [128, 128], F32, kind="ExternalInput")
    if with_qkv_bias:
        bqk_d = nc.dram_tensor("bqk", [128, 1], F32, kind="ExternalInput")
        bv_d = nc.dram_tensor("bv", [DH, 1], F32, kind="ExternalInput")
    y_d = nc.dram_tensor("ypart", [NTOK, DIM], F32, kind="ExternalOutput")

    with tile.TileContext(nc) as tc, ExitStack() as ctx:
        # ---- persistent sbuf ----
        per = ctx.enter_context(tc.tile_pool(name="persist", bufs=1))
        wqk_sb = per.tile([128, NCHUNK, 128], F32, tag="wqk")
        wvz_sb = per.tile([128, NCHUNK, 128], F32, tag="wvz")
        wout_sb = per.tile([DH, DIM], F32, tag="wout")
        ident_sb = per.tile([128, 128], F32, tag="ident")
        mask_sb = per.tile([128, 128], F32, tag="mask")
        ba_sb = per.tile([128, 1], F32, tag="ba")
        nba_sb = per.tile([128, 1], F32, tag="nba")
        ones_sb = per.tile([128, 128], F32, tag="ones")
        sRep = per.tile([128, NTOK], F32, tag="sRep")
        qk_sb = per.tile([128, NTOK], F32, tag="qk")      # rows 0:64 q'T, 64:128 k'T
        v_sb = per.tile([128, NTOK], F32, tag="v")        # rows 0:64 v'; 64:128 zs then k'
        qdup = per.tile([128, NTOK], F32, tag="qdup")     # rows 64:128 q' copy
        # batch-stacked decay pipeline tiles: rows 0:64 = batch0, 64:128 = batch1
        zstk = per.tile([128, N], F32, tag="zstk")    # z*s; later ainv (scan2 out)
        astk = per.tile([128, N], F32, tag="astk")    # sigmoid; later acum_b1 shift
        estk = per.tile([128, N], F32, tag="estk")    # 1+exp(-z); later ainv_b0 shift
        acstk = per.tile([128, N], F32, tag="acstk")  # cumprod(a)

        nc.sync.dma_start(wqk_sb[:], wqk_d[:])
        nc.sync.dma_start(wvz_sb[:], wvz_d[:])
        nc.sync.dma_start(wout_sb[:], wout_d[:])
        nc.sync.dma_start(ident_sb[:], ident_d[:])
        nc.sync.dma_start(mask_sb[:], mask_d[:])
        nc.sync.dma_start(ba_sb[:], ba_d[:])
        nc.sync.dma_start(nba_sb[:], nba_d[:])
        if with_qkv_bias:
            bqk_sb = per.tile([128, 1], F32, tag="bqk")
            bv_sb = per.tile([128, 1], F32, tag="bv")
            nc.sync.dma_start(bqk_sb[:], bqk_d[:])
            nc.sync.dma_start(bv_sb[0:64, :], bv_d[:])
        nc.gpsimd.memset(ones_sb[:], 1.0)
        lsd_sb = per.tile([128, 1], F32, tag="lsd")
        nc.gpsimd.memset(lsd_sb[:], LOG_SQRT_DIM)

        # ---- phase A: load x, sumsq->s, projections ----
        with (
            tc.tile_pool(name="xt", bufs=2) as xtp,
            tc.tile_pool(name="x2", bufs=4) as x2p,
            tc.tile_pool(name="lns", bufs=2) as lnp,
            tc.tile_pool(name="ss_ps", bufs=2, space="PSUM") as ssp,
            tc.tile_pool(name="vz_ps", bufs=2, space="PSUM") as vzp,
            tc.tile_pool(name="qk_ps", bufs=2, space="PSUM") as qkp,
        ):
            for p in range(NPAN):
                cols = bass.ts(p, PANEL)
                xt = []
                for c in range(NCHUNK):
                    xc = xtp.tile([128, PANEL], F32, tag=f"xt{c}")
                    nc.sync.dma_start(xc[:], xT_d[128 * c:128 * (c + 1), cols])
                    xt.append(xc)
                # sum of squares -> replicated on all partitions via all-ones lhsT
                ss_ps = ssp.tile([128, PANEL], F32)
                for c in range(NCHUNK):
                    x2 = x2p.tile([128, PANEL], F32)
                    if c == 0:
                        nc.scalar.square(x2[:], xt[c][:])
                    elif c == 1:
                        nc.vector.tensor_mul(x2[:], xt[c][:], xt[c][:])
                    else:
                        nc.gpsimd.tensor_mul(x2[:], xt[c][:], xt[c][:])
                    nc.tensor.matmul(ss_ps[:], ones_sb[:], x2[:],
                                     start=(c == 0), stop=(c == NCHUNK - 1))
                # s = exp(-0.5*ln(ss) + ln(sqrt(DIM)))  (= sqrt(DIM)/||x_t||)
                lnt = lnp.tile([128, PANEL], F32)
                nc.scalar.activation(lnt[:], ss_ps[:], AF.Ln)
                nc.scalar.activation(sRep[:, cols], lnt[:], AF.Exp,
                                     bias=lsd_sb[:], scale=-0.5)
                # v|z projection, scaled by s at psum->sbuf
                vz_ps = vzp.tile([128, PANEL], F32)
                for c in range(NCHUNK):
                    nc.tensor.matmul(vz_ps[:], wvz_sb[:, c, :], xt[c][:],
                                     start=(c == 0), stop=(c == NCHUNK - 1))
                nc.vector.tensor_mul(v_sb[:, cols], vz_ps[:], sRep[:, cols])
                if with_qkv_bias:
                    nc.vector.tensor_scalar_add(v_sb[0:64, cols], v_sb[0:64, cols],
                                                bv_sb[0:64, :])
                # q|k projection, scaled by s at psum->sbuf
                qk_ps = qkp.tile([128, PANEL], F32)
                for c in range(NCHUNK):
                    nc.tensor.matmul(qk_ps[:], wqk_sb[:, c, :], xt[c][:],
                                     start=(c == 0), stop=(c == NCHUNK - 1))
                nc.vector.tensor_mul(qk_sb[:, cols], qk_ps[:], sRep[:, cols])
                if with_qkv_bias:
                    nc.vector.tensor_scalar_add(qk_sb[:, cols], qk_sb[:, cols],
                                                bqk_sb[:])

        # ---- phase B: decay gate, both batches stacked on the partition axis ----
        H0, H1, FB = slice(0, 64), slice(64, 128), slice(0, N)
        C0, C1 = slice(0, N), slice(N, 2 * N)
        # zstk rows 0:64 = zs(b0), rows 64:128 = zs(b1)
        nc.sync.dma_start(zstk[H0, FB], v_sb[H1, C0])
        nc.sync.dma_start(zstk[H1, FB], v_sb[H1, C1])
        nc.scalar.activation(astk[:], zstk[:], AF.Sigmoid, bias=ba_sb[:])
        nc.scalar.activation(estk[:], zstk[:], AF.Exp, bias=nba_sb[:], scale=-1.0)
        nc.vector.tensor_scalar_add(estk[:], estk[:], 1.0)
        nc.vector.tensor_tensor_scan(acstk[:], astk[:], astk[:], 1.0,
                                     ALU.mult, ALU.bypass)
        # ainv = min(cumprod(1+exp(-z)), 1e8) == 1/max(cumprod(a), 1e-8)
        nc.vector.tensor_tensor_scan(zstk[:], estk[:], estk[:], 1.0,
                                     ALU.mult, ALU.bypass)
        nc.vector.tensor_scalar_min(zstk[:], zstk[:], 1.0 / EPS_INV)
        # partition shifts so each consumer sees its operand on its own lanes
        nc.sync.dma_start(estk[H1, FB], zstk[H0, FB])    # ainv(b0) -> rows 64:
        nc.sync.dma_start(astk[H0, FB], acstk[H1, FB])   # acum(b1) -> rows 0:
        # q' = q * s * a_cum ; k' = k * s * ainv (k' written into v_sb rows 64:
        # so that one PE transpose per block yields both v'tm and k'tm)
        nc.vector.tensor_mul(qk_sb[H0, C0], qk_sb[H0, C0], acstk[H0, FB])
        nc.vector.tensor_mul(qk_sb[H0, C1], qk_sb[H0, C1], astk[H0, FB])
        nc.vector.tensor_mul(v_sb[H1, C0], qk_sb[H1, C0], estk[H1, FB])
        nc.vector.tensor_mul(v_sb[H1, C1], qk_sb[H1, C1], zstk[H1, FB])
        nc.sync.dma_start(qdup[H1, C0], qk_sb[H0, C0])
        nc.sync.dma_start(qdup[H1, C1], qk_sb[H0, C1])

        # ---- phase C: chunked attention + output projection, batches interleaved ----
        with (
            tc.tile_pool(name="vk", bufs=4) as vkp,
            tc.tile_pool(name="ssb", bufs=3) as ssbp,
            tc.tile_pool(name="osb", bufs=3) as osbp,
            tc.tile_pool(name="stsb", bufs=1) as stsbp,
            tc.tile_pool(name="ysb", bufs=3) as ysbp,
            tc.tile_pool(name="psA", bufs=3, space="PSUM") as psA,
            tc.tile_pool(name="psB", bufs=3, space="PSUM") as psB,
            tc.tile_pool(name="psY", bufs=2, space="PSUM") as psY,
        ):
            state_sb = [stsbp.tile([64, 64], F32, tag=f"state{b}",
                                   name=f"state_sb{b}") for b in range(B)]
            for t in range(NBLK):
                for b in range(B):
                    cols = bass.ts(b * NBLK + t, BLK)
                    # one transpose yields [v'tm | k'tm] (v_sb rows: 0:64 v', 64:128 k')
                    tr_ps = psA.tile([128, 128], F32, tag="blk")
                    nc.tensor.transpose(tr_ps[:], v_sb[:, cols], ident_sb[:])
                    vk = vkp.tile([128, 128], F32)
                    if (t + b) % 2 == 0:
                        nc.vector.tensor_copy(vk[:], tr_ps[:])
                    else:
                        nc.scalar.copy(vk[:], tr_ps[:])
                    # S^T = k' q'^T on this block, masked to kt<=qt
                    s_ps = psA.tile([128, BLK], F32, tag="blk")
                    nc.tensor.matmul(s_ps[:], v_sb[64:128, cols], qdup[64:128, cols],
                                     start=True, stop=True)
                    ssb = ssbp.tile([128, BLK], F32)
                    nc.vector.tensor_mul(ssb[:], s_ps[:], mask_sb[:])
                    # O^T = State^T q'^T (inter) + V^T S^T (intra)
                    o_ps = psB.tile([64, BLK], F32, tag="ob")
                    if t > 0:
                        nc.tensor.matmul(o_ps[:], state_sb[b][:], qk_sb[0:64, cols],
                                         start=True, stop=False)
                    nc.tensor.matmul(o_ps[:], vk[:, 0:64], ssb[:],
                                     start=(t == 0), stop=True)
                    # State += K'^T V, accumulated in SBUF
                    if t < NBLK - 1:
                        st_ps = psB.tile([64, 64], F32, tag="ob")
                        nc.tensor.matmul(st_ps[:], vk[:, 64:128], vk[:, 0:64],
                                         start=True, stop=True)
                        if t == 0:
                            nc.vector.tensor_copy(state_sb[b][:], st_ps[:])
                        else:
                            nc.vector.tensor_add(state_sb[b][:], state_sb[b][:],
                                                 st_ps[:])
                    osb = osbp.tile([64, BLK], F32)
                    nc.scalar.copy(osb[:], o_ps[:])
                    # y = O @ wout_h   [128 tok, 512]
                    y_ps = psY.tile([128, DIM], F32)
                    nc.tensor.matmul(y_ps[:], osb[:], wout_sb[:], start=True, stop=True)
                    ysb = ysbp.tile([128, DIM], F32)
                    if (t + b) % 2 == 0:
                        nc.vector.tensor_copy(ysb[:], y_ps[:])
                    else:
                        nc.scalar.copy(ysb[:], y_ps[:])
                    r0 = b * N + t * BLK
                    nc.sync.dma_start(y_d[r0:r0 + BLK, :], ysb[:])
    nc.finalize()
    return nc


_NC_CACHE = {}


def _get_nc(with_qkv_bias: bool):
    if with_qkv_bias not in _NC_CACHE:
        _NC_CACHE[with_qkv_bias] = build_nc(with_qkv_bias)
    return _NC_CACHE[with_qkv_bias]


def make_in_maps(x, gamma, w_qkv, b_qkv, w_a, b_a, w_out, b_out, with_qkv_bias):
    x = np.asarray(x, np.float32)
    gamma = np.asarray(gamma, np.float32)
    w_qkv = np.asarray(w_qkv, np.float32)
    b_qkv = np.asarray(b_qkv, np.float32)
    w_a = np.asarray(w_a, np.float32)
    b_a = np.asarray(b_a, np.float32)

    xT = np.ascontiguousarray(x.reshape(NTOK, DIM).T)
    wq = w_qkv[:, 0:DIM] * gamma[:, None] * SCALE
    wk = w_qkv[:, DIM:2 * DIM] * gamma[:, None]
    wv = w_qkv[:, 2 * DIM:3 * DIM] * gamma[:, None]
    wa = w_a * gamma[:, None]
    ident = np.eye(128, dtype=np.float32)
    mask = np.triu(np.ones((128, 128), np.float32))  # [kt, qt] keep kt<=qt

    in_maps = []
    for h in range(HEADS):
        sl = slice(h * DH, (h + 1) * DH)
        wqk = np.concatenate([wq[:, sl], wk[:, sl]], axis=1)   # [512, 128]
        wvz = np.concatenate([wv[:, sl], wa[:, sl]], axis=1)   # [512, 128]
        m = {
            "xT": xT,
            "wqk": np.ascontiguousarray(wqk.reshape(NCHUNK, 128, 128).transpose(1, 0, 2)),
            "wvz": np.ascontiguousarray(wvz.reshape(NCHUNK, 128, 128).transpose(1, 0, 2)),
            "wout": np.ascontiguousarray(np.asarray(w_out, np.float32)[sl, :]),
            "ba": np.ascontiguousarray(np.tile(b_a[sl], 2)[:, None]),
            "nba": np.ascontiguousarray(np.tile(-b_a[sl], 2)[:, None]),
            "ident": ident,
            "mask": mask,
        }
        if with_qkv_bias:
            bq = b_qkv[0:DIM][sl] * SCALE
            bk = b_qkv[DIM:2 * DIM][sl]
            bv = b_qkv[2 * DIM:3 * DIM][sl]
            m["bqk"] = np.ascontiguousarray(
                np.concatenate([bq, bk])[:, None].astype(np.float32))
            m["bv"] = np.ascontiguousarray(bv[:, None].astype(np.float32))
        in_maps.append(m)
    return in_maps


def kernel(x, gamma, w_qkv, b_qkv, w_a, b_a, w_out, b_out, _profile=None):
    with_qkv_bias = bool(np.any(np.asarray(b_qkv)))
    nc = _get_nc(with_qkv_bias)
    in_maps = make_in_maps(x, gamma, w_qkv, b_qkv, w_a, b_a, w_out, b_out,
                           with_qkv_bias)
    kwargs = dict(_profile) if _profile else {}
    res = run_bass_kernel_spmd(nc, in_maps, core_ids=list(range(HEADS)), **kwargs)
    if _profile is not None:
        _profile["result"] = res
    out = res.results[0]["ypart"].astype(np.float32).copy()
    for i in range(1, HEADS):
        out += res.results[i]["ypart"]
    out += np.asarray(b_out, np.float32)[None, :]
    return out.reshape(B, N, DIM)



# revision 2
# speedup vs baseline: 1.4782x; 1.4782x over previous
"""Trainium2 Bass kernel for nn_CausalFullAttention_13735305413109.

Causal attention with a data-dependent cumprod decay gate and no softmax.
With no softmax the masked quadratic attention is algebraically a chunked
linear attention:
    out_i = q'_i @ State_{blk(i)} + sum_{j<=i, same blk} (q'_i.k'_j) v_j
    State_t = sum_{j < t*BLK} k'_j (x) v_j
with q' = q*SCALE*a_cum, k' = k/max(a_cum,1e-8), per (batch, head).

Sharding: (batch, head-pair) across 8 cores — core c handles batch c//4 and
heads (2*(c%4), 2*(c%4)+1) over that batch's 2048 tokens. Each core emits a
partial out-projection y_part = O_cat @ [w_out[h0]; w_out[h1]] (the in-matmul
sum over its 2 heads); the host sums 4 partials per batch (+ b_out).

All matmuls run in bf16 (1 cycle/row on the PE vs 4 for fp32); the cumprod
scan, decay reciprocal, and state accumulation stay fp32.
"""
import numpy as np
from contextlib import ExitStack

import ml_dtypes
import concourse.bass as bass
import concourse.bacc as bacc
import concourse.mybir as mybir
import concourse.tile as tile
from concourse.bass_utils import run_bass_kernel_spmd

F32 = mybir.dt.float32
BF16 = mybir.dt.bfloat16
AF = mybir.ActivationFunctionType
ALU = mybir.AluOpType

B = 2
N = 2048                # tokens per batch (per core)
DIM = 512
HEADS = 8
DH = 64
BLK = 128               # token block
NBLK = N // BLK         # 16
PANEL = 512
NPAN = N // PANEL       # 4
NCHUNK = DIM // 128     # 4
NGRP = 4                # weight groups: 0=[k0|v0] 1=[k1|v1] 2=[q0|q1] 3=[z0|z1]
SCALE = DH ** -0.5
LOG_SQRT_DIM = float(np.log(np.sqrt(DIM)))
EPS_INV = 1e-8

USE_DMA_TRANSPOSE = True


def build_nc(with_qkv_bias: bool):
    nc = bacc.Bacc()
    x_d = nc.dram_tensor("xT", [128, NCHUNK, N], BF16, kind="ExternalInput")
    w_d = nc.dram_tensor("wall", [128, NCHUNK, NGRP, 128], BF16,
                         kind="ExternalInput")
    wout_d = nc.dram_tensor("wout", [128, DIM], BF16, kind="ExternalInput")
    ba_d = nc.dram_tensor("ba", [128, 1], F32, kind="ExternalInput")
    mask_d = nc.dram_tensor("mask", [128, 128], BF16, kind="ExternalInput")
    ident_d = nc.dram_tensor("ident", [128, 128], BF16, kind="ExternalInput")
    if with_qkv_bias:
        bkv0_d = nc.dram_tensor("bkv0", [128, 1], F32, kind="ExternalInput")
        bkv1_d = nc.dram_tensor("bkv1", [128, 1], F32, kind="ExternalInput")
        bq_d = nc.dram_tensor("bq", [128, 1], F32, kind="ExternalInput")
    y_d = nc.dram_tensor("ypart", [N, DIM], BF16, kind="ExternalOutput")

    with tile.TileContext(nc) as tc, ExitStack() as ctx:
        # ---- persistent sbuf ----
        per = ctx.enter_context(tc.tile_pool(name="persist", bufs=1))
        w_sb = per.tile([128, NCHUNK, NGRP, 128], BF16, tag="wall")
        wout_sb = per.tile([128, DIM], BF16, tag="wout")
        ba_sb = per.tile([128, 1], F32, tag="ba")
        mask_sb = per.tile([128, 128], BF16, tag="mask")
        ident_sb = per.tile([128, 128], BF16, tag="ident")
        ones_sb = per.tile([128, 128], BF16, tag="ones")
        lsd_sb = per.tile([128, 1], F32, tag="lsd")
        # projections / gate, full token range
        GVK0 = per.tile([128, N], BF16, tag="gvk0")   # rows 0:64 k'_h0, 64:128 v'_h0
        GVK1 = per.tile([128, N], BF16, tag="gvk1")
        GQ = per.tile([128, N], BF16, tag="gq")       # rows 0:64 q'_h0, 64:128 q'_h1
        GZ = per.tile([128, N], F32, tag="gz")        # rows [z_h0 | z_h1]
        acum = per.tile([128, N], F32, tag="acum")
        ainv = per.tile([128, N], F32, tag="ainv")
        ainvS = per.tile([64, N], F32, tag="ainvS")   # ainv_h1 moved to lanes 0:64
        QS1 = per.tile([64, N], BF16, tag="qs1")      # q'_h1 moved to lanes 0:64
        # state, both heads stacked: rows 0:64 = head0, 64:128 = head1
        st = per.tile([128, DH], F32, tag="st")
        stb = per.tile([128, DH], BF16, tag="stb")

        nc.sync.dma_start(w_sb[:], w_d[:])
        nc.sync.dma_start(wout_sb[:], wout_d[:])
        nc.sync.dma_start(ba_sb[:], ba_d[:])
        nc.scalar.dma_start(mask_sb[:], mask_d[:])
        nc.scalar.dma_start(ident_sb[:], ident_d[:])
        if with_qkv_bias:
            bkv0_sb = per.tile([128, 1], F32, tag="bkv0")
            bkv1_sb = per.tile([128, 1], F32, tag="bkv1")
            bq_sb = per.tile([128, 1], F32, tag="bq")
            nc.scalar.dma_start(bkv0_sb[:], bkv0_d[:])
            nc.scalar.dma_start(bkv1_sb[:], bkv1_d[:])
            nc.scalar.dma_start(bq_sb[:], bq_d[:])
        nc.gpsimd.memset(ones_sb[:], 1.0)
        nc.gpsimd.memset(lsd_sb[:], LOG_SQRT_DIM)

        with (
            tc.tile_pool(name="xt", bufs=2) as xtp,
            tc.tile_pool(name="x2", bufs=2) as x2p,
            tc.tile_pool(name="sr", bufs=2) as srp,
            tc.tile_pool(name="gat", bufs=2) as gatp,
            tc.tile_pool(name="vkt", bufs=6) as vktp,
            tc.tile_pool(name="ssb", bufs=4) as ssbp,
            tc.tile_pool(name="osb", bufs=3) as osbp,
            tc.tile_pool(name="ysb", bufs=3) as ysbp,
            tc.tile_pool(name="psBig", bufs=3, space="PSUM") as psBig,
            tc.tile_pool(name="psS", bufs=2, space="PSUM") as psS,
            tc.tile_pool(name="psO", bufs=2, space="PSUM") as psO,
            tc.tile_pool(name="psSt", bufs=1, space="PSUM") as psSt,
        ):
            for p in range(NPAN):
                cols = bass.ts(p, PANEL)
                # -- load x chunks --
                xt = []
                for c in range(NCHUNK):
                    xc = xtp.tile([128, PANEL], BF16, tag=f"xt{c}",
                                  name=f"xt{c}_{p}")
                    nc.sync.dma_start(xc[:], x_d[:, c, cols])
                    xt.append(xc)
                # -- sum of squares (raw x) -> s = sqrt(DIM)/||x_t|| --
                x2 = []
                for c in range(NCHUNK):
                    x2c = x2p.tile([128, PANEL], BF16, tag=f"x2{c}",
                                   name=f"x2{c}_{p}")
                    eng = nc.vector if c % 2 == 0 else nc.gpsimd
                    eng.tensor_mul(x2c[:], xt[c][:], xt[c][:])
                    x2.append(x2c)
                nc.vector.tensor_add(x2[0][:], x2[0][:], x2[1][:])
                nc.gpsimd.tensor_add(x2[2][:], x2[2][:], x2[3][:])
                nc.vector.tensor_add(x2[0][:], x2[0][:], x2[2][:])
                ss_ps = psBig.tile([128, PANEL], F32, tag="big",
                                   name=f"ss_ps_{p}")
                nc.tensor.matmul(ss_ps[:], ones_sb[:], x2[0][:],
                                 start=True, stop=True)
                lnt = srp.tile([128, PANEL], F32, tag="lnt", name=f"lnt_{p}")
                sRep = srp.tile([128, PANEL], F32, tag="sRep", name=f"sRep_{p}")
                nc.scalar.activation(lnt[:], ss_ps[:], AF.Ln)
                nc.scalar.activation(sRep[:], lnt[:], AF.Exp,
                                     bias=lsd_sb[:], scale=-0.5)
                # -- scale x in place: x_hat = x * s --
                for c in range(NCHUNK):
                    eng = nc.vector if c % 2 == 0 else nc.gpsimd
                    eng.tensor_mul(xt[c][:], xt[c][:], sRep[:])
                # -- projections: 4 groups, PSUM accum over chunks --
                gdst = [GVK0, GVK1, GQ, GZ]
                for g in range(NGRP):
                    gp = psBig.tile([128, PANEL], F32, tag="big",
                                    name=f"gp{g}_{p}")
                    for c in range(NCHUNK):
                        nc.tensor.matmul(gp[:], w_sb[:, c, g, :], xt[c][:],
                                         start=(c == 0), stop=(c == NCHUNK - 1))
                    nc.scalar.copy(gdst[g][:, cols], gp[:])
                if with_qkv_bias:
                    nc.vector.tensor_scalar_add(GVK0[:, cols], GVK0[:, cols],
                                                bkv0_sb[:])
                    nc.vector.tensor_scalar_add(GVK1[:, cols], GVK1[:, cols],
                                                bkv1_sb[:])
                    nc.vector.tensor_scalar_add(GQ[:, cols], GQ[:, cols],
                                                bq_sb[:])
                # -- decay gate for this panel --
                asig = gatp.tile([128, PANEL], F32, tag="asig",
                                 name=f"asig_{p}")
                amax = gatp.tile([128, PANEL], F32, tag="amax",
                                 name=f"amax_{p}")
                nc.scalar.activation(asig[:], GZ[:, cols], AF.Sigmoid,
                                     bias=ba_sb[:])
                init = 1.0 if p == 0 else acum[:, p * PANEL - 1:p * PANEL]
                nc.vector.tensor_tensor_scan(acum[:, cols], asig[:], asig[:],
                                             init, ALU.mult, ALU.bypass)
                nc.gpsimd.tensor_scalar_max(amax[:], acum[:, cols], EPS_INV)
                nc.vector.reciprocal_approx_fast(ainv[:, cols], amax[:])
                nc.scalar.dma_start(ainvS[:, cols], ainv[64:128, cols])
                # q' = q*s*acum ; k' = k*s*ainv   (in place, bf16 out)
                nc.vector.tensor_mul(GQ[:, cols], GQ[:, cols], acum[:, cols])
                nc.vector.tensor_mul(GVK0[0:64, cols], GVK0[0:64, cols],
                                     ainv[0:64, cols])
                nc.gpsimd.tensor_mul(GVK1[0:64, cols], GVK1[0:64, cols],
                                     ainvS[:, cols])
                nc.scalar.dma_start(QS1[:, cols], GQ[64:128, cols])

                # -- attention over this panel's 4 blocks --
                for tl in range(NPAN):
                    t = NPAN * p + tl
                    bc = bass.ts(t, BLK)
                    o_ps = psO.tile([128, BLK], F32, tag="o", name=f"o_{t}")
                    vkts = []
                    for h in range(2):
                        GVK = GVK0 if h == 0 else GVK1
                        hsl = slice(64 * h, 64 * (h + 1))
                        srhs = GQ[0:64, bc] if h == 0 else QS1[:, bc]
                        # [k' | v'] block -> token-major via DMA xbar
                        vkt = vktp.tile([128, 128], BF16, tag="vkt",
                                        name=f"vkt_{t}_{h}")
                        if USE_DMA_TRANSPOSE:
                            eng = nc.sync if h == 0 else nc.scalar
                            eng.dma_start_transpose(vkt[:], GVK[:, bc])
                        else:
                            tr_ps = psS.tile([128, 128], BF16, tag="s",
                                             name=f"tr_{t}_{h}")
                            nc.tensor.transpose(tr_ps[:], GVK[:, bc],
                                                ident_sb[:])
                            nc.vector.tensor_copy(vkt[:], tr_ps[:])
                        vkts.append(vkt)
                        # S^T = k'^T q' (kt x qt), masked to kt<=qt
                        s_ps = psS.tile([128, BLK], F32, tag="s",
                                        name=f"s_{t}_{h}")
                        nc.tensor.matmul(s_ps[:], GVK[0:64, bc], srhs,
                                         start=True, stop=True)
                        ssb = ssbp.tile([128, BLK], BF16, tag="ssb",
                                        name=f"ssb_{t}_{h}")
                        nc.vector.tensor_mul(ssb[:], s_ps[:], mask_sb[:])
                        # O^T = State^T q' (inter) + V'^T S^T (intra)
                        if t > 0:
                            nc.tensor.matmul(o_ps[hsl, :], stb[hsl, :],
                                             GQ[hsl, bc], start=True,
                                             stop=False)
                        nc.tensor.matmul(o_ps[hsl, :], vkt[:, 64:128], ssb[:],
                                         start=(t == 0), stop=True)
                    # State += K'^T V' for both heads (stacked halves)
                    if t < NBLK - 1:
                        st_ps = psSt.tile([128, DH], F32, tag="st",
                                          name=f"stp_{t}")
                        for h in range(2):
                            hsl = slice(64 * h, 64 * (h + 1))
                            nc.tensor.matmul(st_ps[hsl, :],
                                             vkts[h][:, 0:64],
                                             vkts[h][:, 64:128],
                                             start=True, stop=True)
                        if t == 0:
                            nc.vector.tensor_copy(st[:], st_ps[:])
                        else:
                            nc.vector.tensor_add(st[:], st[:], st_ps[:])
                        nc.scalar.copy(stb[:], st[:])
                    # y = O_cat @ wout_cat  [128 tok, 512]
                    osb = osbp.tile([128, BLK], BF16, tag="osb",
                                    name=f"osb_{t}")
                    nc.scalar.copy(osb[:], o_ps[:])
                    y_ps = psBig.tile([128, DIM], F32, tag="big",
                                      name=f"y_{t}")
                    nc.tensor.matmul(y_ps[:], osb[:], wout_sb[:],
                                     start=True, stop=True)
                    ysb = ysbp.tile([128, DIM], BF16, tag="ysb",
                                    name=f"ysb_{t}")
                    nc.vector.tensor_copy(ysb[:], y_ps[:])
                    eng = nc.sync if t % 2 == 0 else nc.scalar
                    eng.dma_start(y_d[t * BLK:(t + 1) * BLK, :], ysb[:])
    nc.finalize()
    return nc


_NC_CACHE = {}


def _get_nc(with_qkv_bias: bool):
    if with_qkv_bias not in _NC_CACHE:
        _NC_CACHE[with_qkv_bias] = build_nc(with_qkv_bias)
    return _NC_CACHE[with_qkv_bias]


def make_in_maps(x, gamma, w_qkv, b_qkv, w_a, b_a, w_out, b_out, with_qkv_bias):
    x = np.asarray(x, np.float32)
    gamma = np.asarray(gamma, np.float32)
    w_qkv = np.asarray(w_qkv, np.float32)
    b_qkv = np.asarray(b_qkv, np.float32)
    w_a = np.asarray(w_a, np.float32)
    b_a = np.asarray(b_a, np.float32)
    w_out = np.asarray(w_out, np.float32)

    wq = w_qkv[:, 0:DIM] * gamma[:, None] * SCALE
    wk = w_qkv[:, DIM:2 * DIM] * gamma[:, None]
    wv = w_qkv[:, 2 * DIM:3 * DIM] * gamma[:, None]
    wa = w_a * gamma[:, None]
    mask = np.triu(np.ones((128, 128), np.float32))  # [kt, qt] keep kt<=qt
    ident = np.eye(128, dtype=np.float32)

    # xT per batch: [512, N] -> [128, NCHUNK, N], bf16
    xTs = []
    for b in range(B):
        xT = x[b].T.reshape(NCHUNK, 128, N).transpose(1, 0, 2)
        xTs.append(np.ascontiguousarray(xT.astype(ml_dtypes.bfloat16)))

    in_maps = []
    for core in range(HEADS):
        b, pair = divmod(core, B * 2)
        h0, h1 = 2 * pair, 2 * pair + 1
        s0 = slice(h0 * DH, (h0 + 1) * DH)
        s1 = slice(h1 * DH, (h1 + 1) * DH)
        groups = [
            np.concatenate([wk[:, s0], wv[:, s0]], axis=1),
            np.concatenate([wk[:, s1], wv[:, s1]], axis=1),
            np.concatenate([wq[:, s0], wq[:, s1]], axis=1),
            np.concatenate([wa[:, s0], wa[:, s1]], axis=1),
        ]
        # [512, NGRP, 128] -> [128, NCHUNK, NGRP, 128]
        w_all = np.stack(groups, axis=1).reshape(NCHUNK, 128, NGRP, 128)
        w_all = np.ascontiguousarray(
            w_all.transpose(1, 0, 2, 3).astype(ml_dtypes.bfloat16))
        m = {
            "xT": xTs[b],
            "wall": w_all,
            "wout": np.ascontiguousarray(
                np.concatenate([w_out[s0, :], w_out[s1, :]], axis=0)
                .astype(ml_dtypes.bfloat16)),
            "ba": np.ascontiguousarray(
                np.concatenate([b_a[s0], b_a[s1]])[:, None].astype(np.float32)),
            "mask": np.ascontiguousarray(mask.astype(ml_dtypes.bfloat16)),
            "ident": np.ascontiguousarray(ident.astype(ml_dtypes.bfloat16)),
        }
        if with_qkv_bias:
            bq = b_qkv[0:DIM] * SCALE
            bk = b_qkv[DIM:2 * DIM]
            bv = b_qkv[2 * DIM:3 * DIM]
            m["bkv0"] = np.ascontiguousarray(
                np.concatenate([bk[s0], bv[s0]])[:, None].astype(np.float32))
            m["bkv1"] = np.ascontiguousarray(
                np.concatenate([bk[s1], bv[s1]])[:, None].astype(np.float32))
            m["bq"] = np.ascontiguousarray(
                np.concatenate([bq[s0], bq[s1]])[:, None].astype(np.float32))
        in_maps.append(m)
    return in_maps


def kernel(x, gamma, w_qkv, b_qkv, w_a, b_a, w_out, b_out, _profile=None):
    with_qkv_bias = bool(np.any(np.asarray(b_qkv)))
    nc = _get_nc(with_qkv_bias)
    in_maps = make_in_maps(x, gamma, w_qkv, b_qkv, w_a, b_a, w_out, b_out,
                           with_qkv_bias)
    kwargs = dict(_profile) if _profile else {}
    res = run_bass_kernel_spmd(nc, in_maps, core_ids=list(range(HEADS)),
                               **kwargs)
    if _profile is not None:
        _profile["result"] = res
    out = np.zeros((B, N, DIM), np.float32)
    for core in range(HEADS):
        b = core // 4
        out[b] += res.results[core]["ypart"].astype(np.float32)
    out += np.asarray(b_out, np.float32)[None, None, :]
    return out


# revision 12
# speedup vs baseline: 2.2951x; 1.5526x over previous
"""Trainium2 Bass kernel for nn_CausalFullAttention_13735305413109.

Causal attention with a data-dependent cumprod decay gate and no softmax.
With no softmax the masked quadratic attention is algebraically a chunked
linear attention:
    out_i = q'_i @ State_{blk(i)} + sum_{j<=i, same blk} (q'_i.k'_j) v_j
    State_t = sum_{j < t*BLK} k'_j (x) v_j
with q' = q*SCALE*a_cum, k' = k/max(a_cum,1e-8), per (batch, head).

Sharding: (batch, head-pair) across 8 cores — core c handles batch c//4 and
heads (2*(c%4), 2*(c%4)+1) over that batch's 2048 tokens. Each core emits a
partial out-projection y_part = O_cat @ [w_out[h0]; w_out[h1]] (the in-matmul
sum over its 2 heads); the host sums 4 partials per batch (+ b_out).

Numerics: all matmuls in bf16 (1 cyc/row); the decay recurrence is a single
fp32 cumprod scan of (1 + e^-z) = 1/a, so ainv = min(scan, 1e8) matches the
reference 1/max(a_cum, 1e-8) exactly and a_cum = approx_recip(scan). The
rms-norm scale s enters as exp(-0.5 ln(ss)); only exp/ln scalar tables are
used (one table set). v's s-factor is folded into k (k''=k*s^2*ainv, v''=v)
in the no-bias build so v needs no elementwise scaling at all.
"""
import numpy as np
from contextlib import ExitStack

import ml_dtypes
import concourse.bass as bass
import concourse.bacc as bacc
import concourse.mybir as mybir
import concourse.tile as tile
from concourse.bass_utils import run_bass_kernel_spmd

F32 = mybir.dt.float32
BF16 = mybir.dt.bfloat16
AF = mybir.ActivationFunctionType
ALU = mybir.AluOpType

B = 2
N = 2048                # tokens per batch (per core)
DIM = 512
HEADS = 8
DH = 64
BLK = 128
NBLK = N // BLK         # 16
PANEL = 512
NPAN = N // PANEL       # 4
NCHUNK = DIM // 128     # 4
NGRP = 4                # 0=[k0|k1] 1=[v0|v1] 2=[q0|q1] 3=[z0|z1]
SCALE = DH ** -0.5
LOG_SQRT_DIM = float(np.log(np.sqrt(DIM)))
EPS_INV = 1e-8
RAW_CAP = 3e37          # clamp 1/acum before approx reciprocal


def build_nc(with_qkv_bias: bool):
    nc = bacc.Bacc()
    x_d = nc.dram_tensor("xT", [128, NCHUNK, N], BF16, kind="ExternalInput")
    w_d = nc.dram_tensor("wall", [128, NCHUNK, NGRP, 128], BF16,
                         kind="ExternalInput")
    wout_d = nc.dram_tensor("wout", [128, DIM], BF16, kind="ExternalInput")
    nba_d = nc.dram_tensor("nba", [128, 1], F32, kind="ExternalInput")
    mask2_d = nc.dram_tensor("mask2", [128, 256], BF16, kind="ExternalInput")
    ident_d = nc.dram_tensor("ident", [128, 128], BF16, kind="ExternalInput")
    if with_qkv_bias:
        bk_d = nc.dram_tensor("bk", [128, 1], F32, kind="ExternalInput")
        bv_d = nc.dram_tensor("bv", [128, 1], F32, kind="ExternalInput")
        bq_d = nc.dram_tensor("bq", [128, 1], F32, kind="ExternalInput")
    y_d = nc.dram_tensor("ypart", [N, DIM], BF16, kind="ExternalOutput")

    with tile.TileContext(nc) as tc, ExitStack() as ctx:
        per = ctx.enter_context(tc.tile_pool(name="persist", bufs=1))
        xsb = per.tile([128, NCHUNK, N], BF16, tag="xsb")
        w_sb = per.tile([128, NCHUNK, NGRP, 128], BF16, tag="wall")
        wout_sb = per.tile([128, DIM], BF16, tag="wout")
        nba_sb = per.tile([128, 1], F32, tag="nba")
        mask2_sb = per.tile([128, 256], BF16, tag="mask2")
        ident_sb = per.tile([128, 128], BF16, tag="ident")
        ones_sb = per.tile([128, 128], BF16, tag="ones")
        lsd_sb = per.tile([128, 1], F32, tag="lsd")
        cap_sb = per.tile([128, PANEL], F32, tag="cap")
        sRep = per.tile([128, N], F32, tag="sRep")    # sqrt(DIM)/||x_t||
        sRep2 = per.tile([128, N], F32, tag="sRep2")  # 1/ss (DIM folded in wk)
        GK = per.tile([128, N], BF16, tag="gk")       # rows [k_h0 | k_h1]
        GV = per.tile([128, N], BF16, tag="gv")
        GQ = per.tile([128, N], BF16, tag="gq")
        GZ = per.tile([128, N], F32, tag="gz")
        araw = per.tile([128, N], F32, tag="araw")    # cumprod(1/a) = 1/acum
        ainvb = per.tile([128, N], BF16, tag="ainvb")  # min(araw, 1e8)
        st = per.tile([128, DH], F32, tag="st")       # state accum (fp32)
        stb = per.tile([128, DH], BF16, tag="stb")    # state, bf16 snapshot

        nc.sync.dma_start(w_sb[:], w_d[:])
        nc.sync.dma_start(wout_sb[:], wout_d[:])
        nc.sync.dma_start(nba_sb[:], nba_d[:])
        nc.scalar.dma_start(mask2_sb[:], mask2_d[:])
        nc.scalar.dma_start(ident_sb[:], ident_d[:])
        if with_qkv_bias:
            bk_sb = per.tile([128, 1], F32, tag="bk")
            bv_sb = per.tile([128, 1], F32, tag="bv")
            bq_sb = per.tile([128, 1], F32, tag="bq")
            nc.scalar.dma_start(bk_sb[:], bk_d[:])
            nc.scalar.dma_start(bv_sb[:], bv_d[:])
            nc.scalar.dma_start(bq_sb[:], bq_d[:])
        nc.gpsimd.memset(ones_sb[:], 1.0)
        nc.gpsimd.memset(lsd_sb[:], LOG_SQRT_DIM)
        nc.gpsimd.memset(cap_sb[:], RAW_CAP)

        with (
            tc.tile_pool(name="x2", bufs=2) as x2p,
            tc.tile_pool(name="lnt", bufs=4) as lntp,
            tc.tile_pool(name="gat", bufs=2) as gatp,
            tc.tile_pool(name="vkt", bufs=4) as vktp,
            tc.tile_pool(name="ssb", bufs=4) as ssbp,
            tc.tile_pool(name="osb", bufs=3) as osbp,
            tc.tile_pool(name="ysb", bufs=3) as ysbp,
            tc.tile_pool(name="psBig", bufs=2, space="PSUM") as psBig,
            tc.tile_pool(name="psTR", bufs=2, space="PSUM") as psTR,
            tc.tile_pool(name="psSP", bufs=2, space="PSUM") as psSP,
            tc.tile_pool(name="psO", bufs=1, space="PSUM") as psO,
            tc.tile_pool(name="psSt", bufs=1, space="PSUM") as psSt,
        ):
            # ---- x load + sum-of-squares for all panels (Ln's batched) ----
            for p in range(NPAN):
                cols = bass.ts(p, PANEL)
                for c in range(NCHUNK):
                    eng = nc.sync if (p + c) % 2 == 0 else nc.scalar
                    eng.dma_start(xsb[:, c, cols], x_d[:, c, cols])
            lnts = []
            for p in range(NPAN):
                cols = bass.ts(p, PANEL)
                x2 = []
                for c in range(NCHUNK):
                    x2c = x2p.tile([128, PANEL], BF16, tag=f"x2{c}",
                                   name=f"x2{c}_{p}")
                    eng = nc.vector if c % 2 == 0 else nc.gpsimd
                    eng.tensor_mul(x2c[:], xsb[:, c, cols], xsb[:, c, cols])
                    x2.append(x2c)
                ss_ps = psBig.tile([128, PANEL], F32, tag="big",
                                   name=f"ss_{p}")
                for c in range(NCHUNK):
                    nc.tensor.matmul(ss_ps[:], ones_sb[:], x2[c][:],
                                     start=(c == 0), stop=(c == NCHUNK - 1))
                lnt = lntp.tile([128, PANEL], F32, tag="lnt", name=f"lnt_{p}")
                nc.scalar.activation(lnt[:], ss_ps[:], AF.Ln)
                lnts.append(lnt)
            # all Exp's together (same table set as Ln)
            for p in range(NPAN):
                cols = bass.ts(p, PANEL)
                nc.scalar.activation(sRep[:, cols], lnts[p][:], AF.Exp,
                                     bias=lsd_sb[:], scale=-0.5)
                nc.scalar.activation(sRep2[:, cols], lnts[p][:], AF.Exp,
                                     scale=-1.0)

            # ---- per panel: projections, gate, attention ----
            for p in range(NPAN):
                cols = bass.ts(p, PANEL)
                gdst = [GK, GV, GQ, GZ]
                for g in range(NGRP):
                    gp = psBig.tile([128, PANEL], F32, tag="big",
                                    name=f"gp{g}_{p}")
                    for c in range(NCHUNK):
                        nc.tensor.matmul(gp[:], w_sb[:, c, g, :],
                                         xsb[:, c, cols],
                                         start=(c == 0), stop=(c == NCHUNK - 1))
                    dst = gdst[g][:, cols]
                    if g == 0:    # k: * s^2 (no-bias) or * s (bias build)
                        sc = sRep[:, cols] if with_qkv_bias else sRep2[:, cols]
                        nc.vector.tensor_mul(dst, gp[:], sc)
                        if with_qkv_bias:
                            nc.vector.tensor_scalar_add(dst, dst, bk_sb[:])
                    elif g == 1:  # v: unscaled (no-bias) or * s (bias build)
                        if with_qkv_bias:
                            nc.vector.tensor_mul(dst, gp[:], sRep[:, cols])
                            nc.vector.tensor_scalar_add(dst, dst, bv_sb[:])
                        else:
                            nc.scalar.copy(dst, gp[:])
                    elif g == 2:  # q: * s (SCALE folded in weights)
                        nc.vector.tensor_mul(dst, gp[:], sRep[:, cols])
                        if with_qkv_bias:
                            nc.vector.tensor_scalar_add(dst, dst, bq_sb[:])
                    else:         # z: * s, fp32
                        nc.vector.tensor_mul(dst, gp[:], sRep[:, cols])

                # gate: araw = cumprod(1 + e^-(z+ba)) = 1/a_cum
                u = gatp.tile([128, PANEL], F32, tag="u", name=f"u_{p}")
                nc.scalar.activation(u[:], GZ[:, cols], AF.Exp,
                                     bias=nba_sb[:], scale=-1.0)
                nc.vector.tensor_scalar_add(u[:], u[:], 1.0)
                init = 1.0 if p == 0 else araw[:, p * PANEL - 1:p * PANEL]
                # state = min(w*state, 3e37): capped 1/a_cum, never inf
                nc.vector.tensor_tensor_scan(araw[:, cols], u[:], cap_sb[:],
                                             init, ALU.mult, ALU.min)
                nc.vector.tensor_scalar_min(ainvb[:, cols], araw[:, cols],
                                            1.0 / EPS_INV)
                acum = gatp.tile([128, PANEL], F32, tag="acum", name=f"ac_{p}")
                nc.vector.reciprocal_approx_fast(acum[:], araw[:, cols])
                nc.gpsimd.tensor_mul(GK[:, cols], GK[:, cols], ainvb[:, cols])
                nc.vector.tensor_mul(GQ[:, cols], GQ[:, cols], acum[:])

                # attention over this panel's 4 blocks
                for tl in range(PANEL // BLK):
                    t = 4 * p + tl
                    bc = bass.ts(t, BLK)
                    vkT = vktp.tile([128, 256], BF16, tag="vkt",
                                    name=f"vkt_{t}")
                    for h in range(2):
                        tr_ps = psTR.tile([128, 128], BF16, tag="tr",
                                          name=f"tr_{t}_{h}")
                        nc.tensor.transpose(tr_ps[:], (GK if h == 0 else GV)[:, bc],
                                            ident_sb[:])
                        nc.scalar.copy(vkT[:, 128 * h:128 * (h + 1)], tr_ps[:])
                    ssb = ssbp.tile([128, 256], BF16, tag="ssb",
                                    name=f"ssb_{t}")
                    for h in range(2):
                        hsl = slice(64 * h, 64 * (h + 1))
                        sp = psSP.tile([128, BLK], F32, tag="sp",
                                       name=f"sp_{t}_{h}")
                        nc.tensor.matmul(sp[:], GK[hsl, bc], GQ[hsl, bc],
                                         start=True, stop=True)
                        nc.vector.tensor_mul(ssb[:, 128 * h:128 * (h + 1)],
                                             sp[:], mask2_sb[:, 0:128])
                    o_ps = psO.tile([128, BLK], F32, tag="o", name=f"o_{t}")
                    for h in range(2):
                        hsl = slice(64 * h, 64 * (h + 1))
                        if t > 0:
                            nc.tensor.matmul(o_ps[hsl, :], stb[hsl, :],
                                             GQ[hsl, bc], start=True,
                                             stop=False)
                        nc.tensor.matmul(o_ps[hsl, :],
                                         vkT[:, 128 + 64 * h:192 + 64 * h],
                                         ssb[:, 128 * h:128 * (h + 1)],
                                         start=(t == 0), stop=True)
                    if t < NBLK - 1:
                        st_ps = psSt.tile([128, DH], F32, tag="st",
                                          name=f"st_{t}")
                        for h in range(2):
                            hsl = slice(64 * h, 64 * (h + 1))
                            nc.tensor.matmul(st_ps[hsl, :],
                                             vkT[:, 64 * h:64 * (h + 1)],
                                             vkT[:, 128 + 64 * h:192 + 64 * h],
                                             start=True, stop=True)
                        if t == 0:
                            nc.vector.tensor_copy(st[:], st_ps[:])
                        else:
                            nc.vector.tensor_add(st[:], st[:], st_ps[:])
                        nc.scalar.copy(stb[:], st[:])
                    osb = osbp.tile([128, BLK], BF16, tag="osb",
                                    name=f"osb_{t}")
                    nc.scalar.copy(osb[:], o_ps[:])
                    y_ps = psBig.tile([128, DIM], F32, tag="big",
                                      name=f"y_{t}")
                    nc.tensor.matmul(y_ps[:], osb[:], wout_sb[:],
                                     start=True, stop=True)
                    ysb = ysbp.tile([128, DIM], BF16, tag="ysb",
                                    name=f"ysb_{t}")
                    nc.scalar.copy(ysb[:], y_ps[:])
                    eng = nc.sync if t % 2 == 0 else nc.scalar
                    eng.dma_start(y_d[t * BLK:(t + 1) * BLK, :], ysb[:])
    nc.finalize()
    return nc


_NC_CACHE = {}


def _get_nc(with_qkv_bias: bool):
    if with_qkv_bias not in _NC_CACHE:
        _NC_CACHE[with_qkv_bias] = build_nc(with_qkv_bias)
    return _NC_CACHE[with_qkv_bias]


def make_in_maps(x, gamma, w_qkv, b_qkv, w_a, b_a, w_out, b_out, with_qkv_bias):
    x = np.asarray(x, np.float32)
    gamma = np.asarray(gamma, np.float32)
    w_qkv = np.asarray(w_qkv, np.float32)
    b_qkv = np.asarray(b_qkv, np.float32)
    w_a = np.asarray(w_a, np.float32)
    b_a = np.asarray(b_a, np.float32)
    w_out = np.asarray(w_out, np.float32)

    wq = w_qkv[:, 0:DIM] * gamma[:, None] * SCALE
    wk = w_qkv[:, DIM:2 * DIM] * gamma[:, None]
    if not with_qkv_bias:
        wk = wk * float(DIM)  # with s^2 = DIM/ss folded: k'' = k * DIM/ss
    wv = w_qkv[:, 2 * DIM:3 * DIM] * gamma[:, None]
    wa = w_a * gamma[:, None]
    mask = np.triu(np.ones((128, 128), np.float32))  # [kt, qt] keep kt<=qt
    mask2 = np.concatenate([mask, mask], axis=1)
    ident = np.eye(128, dtype=np.float32)

    xTs = []
    for b in range(B):
        xT = x[b].T.reshape(NCHUNK, 128, N).transpose(1, 0, 2)
        xTs.append(np.ascontiguousarray(xT.astype(ml_dtypes.bfloat16)))

    in_maps = []
    for core in range(HEADS):
        b, pair = divmod(core, 4)
        h0, h1 = 2 * pair, 2 * pair + 1
        s0 = slice(h0 * DH, (h0 + 1) * DH)
        s1 = slice(h1 * DH, (h1 + 1) * DH)
        groups = [
            np.concatenate([wk[:, s0], wk[:, s1]], axis=1),
            np.concatenate([wv[:, s0], wv[:, s1]], axis=1),
            np.concatenate([wq[:, s0], wq[:, s1]], axis=1),
            np.concatenate([wa[:, s0], wa[:, s1]], axis=1),
        ]
        w_all = np.stack(groups, axis=1).reshape(NCHUNK, 128, NGRP, 128)
        w_all = np.ascontiguousarray(
            w_all.transpose(1, 0, 2, 3).astype(ml_dtypes.bfloat16))
        m = {
            "xT": xTs[b],
            "wall": w_all,
            "wout": np.ascontiguousarray(
                np.concatenate([w_out[s0, :], w_out[s1, :]], axis=0)
                .astype(ml_dtypes.bfloat16)),
            "nba": np.ascontiguousarray(
                -np.concatenate([b_a[s0], b_a[s1]])[:, None].astype(np.float32)),
            "mask2": np.ascontiguousarray(mask2.astype(ml_dtypes.bfloat16)),
            "ident": np.ascontiguousarray(ident.astype(ml_dtypes.bfloat16)),
        }
        if with_qkv_bias:
            bq = b_qkv[0:DIM] * SCALE
            bk = b_qkv[DIM:2 * DIM]
            bv = b_qkv[2 * DIM:3 * DIM]
            m["bk"] = np.ascontiguousarray(
                np.concatenate([bk[s0], bk[s1]])[:, None].astype(np.float32))
            m["bv"] = np.ascontiguousarray(
                np.concatenate([bv[s0], bv[s1]])[:, None].astype(np.float32))
            m["bq"] = np.ascontiguousarray(
                np.concatenate([bq[s0], bq[s1]])[:, None].astype(np.float32))
        in_maps.append(m)
    return in_maps


def kernel(x, gamma, w_qkv, b_qkv, w_a, b_a, w_out, b_out, _profile=None):
    with_qkv_bias = bool(np.any(np.asarray(b_qkv)))
    nc = _get_nc(with_qkv_bias)
    in_maps = make_in_maps(x, gamma, w_qkv, b_qkv, w_a, b_a, w_out, b_out,
                           with_qkv_bias)
    kwargs = dict(_profile) if _profile else {}
    res = run_bass_kernel_spmd(nc, in_maps, core_ids=list(range(HEADS)),
                               **kwargs)
    if _profile is not None:
        _profile["result"] = res
    out = np.zeros((B, N, DIM), np.float32)
    for core in range(HEADS):
        out[core // 4] += res.results[core]["ypart"].astype(np.float32)
    out += np.asarray(b_out, np.float32)[None, None, :]
    return out
